# revision 9
# baseline (speedup 1.0000x reference)
"""Trainium2 Bass kernel for CollapsedPBFA (collapsed Chebyshev linear attention).

Full-input contract: kernel(x, W_in, W_out) -> (B, S, D) float32.

Sharding: B x H = 2 x 16 = 32 (batch, head) pairs; each of the 8 cores owns
one batch element's 4-head block (cores 0-3 -> b=0, cores 4-7 -> b=1).
QKV projection is column-parallel per head block; the output projection is
row-parallel and the host sums the per-core partials per batch element.

Structure (per 128-row s-tile, software-pipelined front/back issue order):
  front(i): QKV matmuls -> feature chain (f3 = T3/2, g5 = T5/2 stored, the
    2^a factors folded into the beta' cumsum stationaries / den consts) ->
    per-head feature sums -> Tv -> triangular-matmul causal cumsum with per-p
    carry matmuls -> single scalar-engine evac of the 3-bank psum prefix.
  back(i): prods -> p-reduction -> den chain -> out_h -> DMA-xbar transpose
    -> output projection (two 512-wide passes) -> DMA out.
"""

import sys

for _p in ("/opt/trn_rl_repo", "/root/.axon_site/_ro/trn_rl_repo"):
    if _p not in sys.path:
        sys.path.append(_p)

import numpy as np

import concourse.bacc as bacc
import concourse.bass as bass
import concourse.tile as tile
from concourse import mybir

F32 = mybir.dt.float32
BF16 = mybir.dt.bfloat16

B, S, D = 2, 1024, 1024
H, DH = 16, 64
HPC = 4                    # heads per core
EC = HPC * DH              # 256 feature cols per core side
NP = 5                     # stored Chebyshev orders 1..5 (f3, g5 halved)
NS = S // 128              # 8 s-tiles
NKD = D // 128             # 8 k-tiles over d for QKV
EPS_DEN = 1e-7
INV_SQRT_D = 1.0 / 8.0     # 1/sqrt(64)
KV = NP * EC               # 1280 kv channels
DEN0 = KV                  # den channels at [1280:1300] of kvt
NDEN = NP * HPC            # 20


def _beta():
    j = np.arange(6, dtype=np.float32)
    alpha = (j + 1.0) ** (-1.5)
    tail = np.flip(np.cumsum(np.flip(alpha)))
    beta = np.concatenate([np.zeros(1, np.float32), tail[1:].astype(np.float32),
                           np.zeros(5, np.float32)])
    return beta / beta.sum()          # (11,); nonzero at 1..5


# beta' with the stored-feature scale folded in (f3 = T3/2, g5 = T5/2)
_BSCALE = np.array([1.0, 1.0, 4.0, 1.0, 4.0], np.float32)


def _mid_bcast(ap, reps, at=1):
    """Insert a stride-0 dim of length `reps` into an AP's free dims."""
    new = list(ap.ap)
    new.insert(at, [0, reps])
    return bass.AP(tensor=ap.tensor, offset=ap.offset, ap=new)


def _build():
    nc = bacc.Bacc("TRN2", target_bir_lowering=False, debug=False, num_devices=8)

    XT = nc.dram_tensor("xt", [D, S], BF16, kind="ExternalInput")
    WQKVT = nc.dram_tensor("wqkvt", [D, 3 * EC], BF16, kind="ExternalInput")
    WOUTT = nc.dram_tensor("woutt", [EC, D], BF16, kind="ExternalInput")
    LTB = nc.dram_tensor("ltb", [6, 128, 128], BF16, kind="ExternalInput")
    BCONST = nc.dram_tensor("bconst", [128, NDEN], F32, kind="ExternalInput")
    PART = nc.dram_tensor("part", [S, D], F32, kind="ExternalOutput")

    OP = mybir.AluOpType
    AX = mybir.AxisListType
    ACT = mybir.ActivationFunctionType

    with tile.TileContext(nc) as tc:
        with (
            nc.allow_low_precision(reason="bf16 feature pipeline by design"),
            tc.tile_pool(name="persist", bufs=1) as pp,
            tc.tile_pool(name="work", bufs=4) as wp,
            tc.tile_pool(name="ps_qkv", bufs=2, space="PSUM") as ps_qkv,
            tc.tile_pool(name="ps_kv", bufs=1, space="PSUM") as ps_kv,
            tc.tile_pool(name="ps_o", bufs=1, space="PSUM") as ps_o,
        ):
            xt = pp.tile([128, NKD, S], BF16)
            wqkvt = pp.tile([128, NKD, 3 * EC], BF16)
            woutt = pp.tile([128, 2, D], BF16)
            ltb = pp.tile([128, 6, 128], BF16)
            bconst = pp.tile([128, NDEN], F32)
            ones1 = pp.tile([1, 128], BF16)

            # interleave weight/x chunk loads so QKV can start early
            for k in range(NKD):
                nc.sync.dma_start(out=wqkvt[:, k, :], in_=WQKVT[128 * k:128 * (k + 1), :])
                nc.scalar.dma_start(out=xt[:, k, :], in_=XT[128 * k:128 * (k + 1), :])
            for k in range(2):
                nc.scalar.dma_start(out=woutt[:, k, :], in_=WOUTT[128 * k:128 * (k + 1), :])
            for p in range(6):
                nc.sync.dma_start(out=ltb[:, p, :], in_=LTB[p])
            nc.sync.dma_start(out=bconst, in_=BCONST.ap())
            nc.vector.memset(ones1, 1.0)

            st = [None] * NS   # per-tile state for the back half
            kvt_prev = None

            def front(i):
                nonlocal kvt_prev
                si = slice(128 * i, 128 * (i + 1))
                first = (i == 0)

                # QKV projection: psum [q 0:256 | k 256:512 | v 512:768]
                qkv = ps_qkv.tile([128, 768], F32, tag="qkv")
                for k in range(NKD):
                    lhs = xt[:, k, si]
                    nc.tensor.matmul(qkv[:, 0:512], lhs, wqkvt[:, k, 0:512],
                                     start=(k == 0), stop=(k == NKD - 1))
                    nc.tensor.matmul(qkv[:, 512:768], lhs, wqkvt[:, k, 512:768],
                                     start=(k == 0), stop=(k == NKD - 1))

                # feature chain; fqk [p, q 0:256 | k 256:512]
                fqk = wp.tile([128, NP, 512], BF16, tag="fqk")
                m2 = wp.tile([128, 512], BF16, tag="m2")
                m4 = wp.tile([128, 512], BF16, tag="m4")
                u5 = wp.tile([128, 512], BF16, tag="u5")
                vt = wp.tile([128, EC], BF16, tag="vt")
                nc.scalar.copy(out=fqk[:, 0, :], in_=qkv[:, 0:512])
                nc.scalar.activation(out=m2, in_=qkv[:, 0:512], func=ACT.Square)
                nc.vector.tensor_copy(out=vt, in_=qkv[:, 512:768])
                nc.vector.tensor_scalar(out=fqk[:, 1, :], in0=m2,
                                        scalar1=2.0, scalar2=-1.0,
                                        op0=OP.mult, op1=OP.add)
                nc.vector.scalar_tensor_tensor(out=fqk[:, 2, :], in0=fqk[:, 1, :],
                                               scalar=-0.5, in1=fqk[:, 0, :],
                                               op0=OP.add, op1=OP.mult)
                nc.scalar.activation(out=m4, in_=fqk[:, 1, :], func=ACT.Square)
                nc.vector.tensor_scalar(out=fqk[:, 3, :], in0=m4,
                                        scalar1=2.0, scalar2=-1.0,
                                        op0=OP.mult, op1=OP.add)
                # T5 = 2x*T4 - T3 -> g5 = x*t4 - f3 = T5/2
                nc.gpsimd.tensor_tensor(out=u5, in0=fqk[:, 0, :],
                                        in1=fqk[:, 3, :], op=OP.mult)
                nc.gpsimd.tensor_tensor(out=fqk[:, 4, :], in0=u5,
                                        in1=fqk[:, 2, :], op=OP.subtract)

                # k-side per-head sums (cumsum den channels)
                sums = wp.tile([128, 2 * NDEN], BF16, tag="sums")
                st[i] = [fqk, sums, None, si]
                nc.vector.tensor_reduce(
                    out=sums[:, NDEN:2 * NDEN].rearrange("a (p h) -> a p h", p=NP),
                    in_=fqk[:, :, 256:512].rearrange("a p (h d) -> a p h d", h=HPC),
                    axis=AX.X, op=OP.add)

                # Tv = Tk * v
                tv = wp.tile([128, NP, EC], BF16, tag="tv")
                nc.gpsimd.tensor_tensor(out=tv, in0=fqk[:, :, 256:512],
                                        in1=_mid_bcast(vt, NP), op=OP.mult)
                st[i].append(tv)

            def mid(i):
                nonlocal kvt_prev
                fqk, sums, _, si, tv = st[i]
                first = (i == 0)

                # causal cumsum: bank0 [p0 p1] bank1 [p2 p3] bank2 [p4|den]
                kv = ps_kv.tile([128, 3, 512], F32, tag="kv")
                for p in range(NP):
                    dst = kv[:, p // 2, 256 * (p % 2):256 * (p % 2) + 256]
                    nc.tensor.matmul(dst, ltb[:, p, :], tv[:, p, :],
                                     start=True, stop=first,
                                     skip_group_check=True)
                dend = kv[:, 2, 256:256 + NDEN]
                nc.tensor.matmul(dend, ltb[:, 5, :], sums[:, NDEN:2 * NDEN],
                                 start=True, stop=first, skip_group_check=True)
                if not first:
                    # bank-wide carry accumulate (row 0 of previous evac)
                    for bk, (lo, w) in enumerate(((0, 512), (512, 512),
                                                  (1024, 276))):
                        nc.tensor.matmul(kv[:, bk, 0:w], ones1,
                                         kvt_prev[0:1, lo:lo + w],
                                         start=False, stop=True,
                                         skip_group_check=True)

                # single evac of kv prefix + den prefix (row 0 = next carry)
                kvt = wp.tile([128, 1300], BF16, tag="kvt")
                nc.scalar.copy(out=kvt,
                               in_=kv.rearrange("a b c -> a (b c)")[:, 0:1300])
                kvt_prev = kvt
                st[i][2] = kvt

            def back(i):
                fqk, sums, kvt, si, _ = st[i]
                st[i] = None

                # q-side per-head sums (for den)
                nc.vector.tensor_reduce(
                    out=sums[:, 0:NDEN].rearrange("a (p h) -> a p h", p=NP),
                    in_=fqk[:, :, 0:256].rearrange("a p (h d) -> a p h d", h=HPC),
                    axis=AX.X, op=OP.add)

                # num: prods then reduce over p
                prods = wp.tile([128, EC, NP], BF16, tag="prods")
                numq = wp.tile([128, EC], F32, tag="numq")
                nc.gpsimd.tensor_tensor(
                    out=prods.rearrange("a e p -> a p e"), in0=fqk[:, :, 0:256],
                    in1=kvt[:, 0:KV].rearrange("a (p e) -> a p e", p=NP),
                    op=OP.mult)
                nc.vector.tensor_reduce(
                    out=numq, in_=prods, axis=AX.X, op=OP.add)

                # den chain
                qsb = wp.tile([128, NDEN], BF16, tag="qsb")
                dpr = wp.tile([128, NDEN], F32, tag="dpr")
                den4 = wp.tile([128, HPC], F32, tag="den4")
                den4e = wp.tile([128, HPC], F32, tag="den4e")
                rden = wp.tile([128, HPC], F32, tag="rden")
                nc.vector.scalar_tensor_tensor(out=qsb, in0=sums[:, 0:NDEN],
                                               scalar=1.0, in1=bconst,
                                               op0=OP.mult, op1=OP.mult)
                nc.vector.scalar_tensor_tensor(out=dpr, in0=qsb, scalar=1.0,
                                               in1=kvt[:, DEN0:DEN0 + NDEN],
                                               op0=OP.mult, op1=OP.mult)
                nc.vector.tensor_reduce(
                    out=den4, in_=dpr.rearrange("a (p h) -> a h p", p=NP),
                    axis=AX.X, op=OP.add)
                nc.vector.tensor_scalar_add(out=den4e, in0=den4, scalar1=EPS_DEN)
                nc.vector.reciprocal(out=rden, in_=den4e)

                # out_h = num * rden (rden broadcast over Dh)
                outh = wp.tile([128, EC], BF16, tag="outh")
                nc.vector.tensor_tensor(
                    out=outh.rearrange("a (h d) -> a h d", h=HPC),
                    in0=numq.rearrange("a (h d) -> a h d", h=HPC),
                    in1=_mid_bcast(rden, DH, at=2), op=OP.mult)

                # transpose via DMA xbar, then two 512-wide projection passes
                outt = wp.tile([128, 2, 128], BF16, tag="outt")
                nc.sync.dma_start_transpose(out=outt, in_=outh)
                for n in range(2):
                    po = ps_o.tile([128, 512], F32, tag="po")
                    for kt in range(2):
                        nc.tensor.matmul(po, outt[:, kt, :],
                                         woutt[:, kt, 512 * n:512 * (n + 1)],
                                         start=(kt == 0), stop=(kt == 1))
                    outfull = wp.tile([128, 512], F32, tag="outfull")
                    nc.scalar.copy(out=outfull, in_=po)
                    nc.sync.dma_start(out=PART[si, 512 * n:512 * (n + 1)],
                                      in_=outfull)

            for i in range(NS + 2):
                if i < NS:
                    front(i)
                if 0 <= i - 2 < NS:
                    back(i - 2)
                if 0 <= i - 1 < NS:
                    mid(i - 1)

    nc.compile()
    return nc


_NC = None


def _get_nc():
    global _NC
    if _NC is None:
        _NC = _build()
    return _NC


def _stage_inputs(x, W_in, W_out):
    import ml_dtypes
    bf = ml_dtypes.bfloat16
    beta = _beta()
    bprime = beta[1:6] * _BSCALE          # (5,)
    tri = np.tril(np.ones((128, 128), np.float32))
    ltb = np.stack([bprime[p] * tri for p in range(5)] + [tri]).astype(bf)
    bconst = np.broadcast_to(np.repeat(bprime, HPC)[None, :],
                             (128, NDEN)).astype(np.float32).copy()
    in_maps = []
    for c in range(8):
        b, hb = divmod(c, 4)
        rs = slice(256 * hb, 256 * (hb + 1))
        wq = W_in[0 * D + 256 * hb:0 * D + 256 * (hb + 1)] * INV_SQRT_D
        wk = W_in[1 * D + 256 * hb:1 * D + 256 * (hb + 1)] * INV_SQRT_D
        wv = W_in[2 * D + 256 * hb:2 * D + 256 * (hb + 1)]
        wqkvt = np.ascontiguousarray(
            np.concatenate([wq, wk, wv], axis=0).T).astype(bf)
        xrev = x[b].T.reshape(D, NS, 128)[:, :, ::-1].reshape(D, S)
        in_maps.append({
            "xt": np.ascontiguousarray(xrev).astype(bf),
            "wqkvt": wqkvt,
            "woutt": np.ascontiguousarray(W_out[:, rs].T).astype(bf),
            "ltb": ltb,
            "bconst": bconst,
        })
    return in_maps


def kernel(x, W_in, W_out):
    from concourse.bass_utils import run_bass_kernel_spmd

    x = np.asarray(x, dtype=np.float32)
    W_in = np.asarray(W_in, dtype=np.float32)
    W_out = np.asarray(W_out, dtype=np.float32)
    nc = _get_nc()
    in_maps = _stage_inputs(x, W_in, W_out)
    res = run_bass_kernel_spmd(nc, in_maps, core_ids=list(range(8)))
    out = np.zeros((B, S, D), dtype=np.float32)
    for c in range(8):
        part = res.results[c]["part"].reshape(NS, 128, D)[:, ::-1, :].reshape(S, D)
        out[c // 4] += part
    return out


# revision 10
# speedup vs baseline: 1.0046x; 1.0046x over previous
"""Trainium2 Bass kernel for CollapsedPBFA (collapsed Chebyshev linear attention).

Full-input contract: kernel(x, W_in, W_out) -> (B, S, D) float32.

Sharding: B x H = 2 x 16 = 32 (batch, head) pairs; each of the 8 cores owns
one batch element's 4-head block (cores 0-3 -> b=0, cores 4-7 -> b=1).
QKV projection is column-parallel per head block; the output projection is
row-parallel and the host sums the per-core partials per batch element.

Structure (per 128-row s-tile, software-pipelined front/back issue order):
  front(i): QKV matmuls -> feature chain (f3 = T3/2, g5 = T5/2 stored, the
    2^a factors folded into the beta' cumsum stationaries / den consts) ->
    per-head feature sums -> Tv -> triangular-matmul causal cumsum with per-p
    carry matmuls -> single scalar-engine evac of the 3-bank psum prefix.
  back(i): prods -> p-reduction -> den chain -> out_h -> DMA-xbar transpose
    -> output projection (two 512-wide passes) -> DMA out.
"""

import sys

for _p in ("/opt/trn_rl_repo", "/root/.axon_site/_ro/trn_rl_repo"):
    if _p not in sys.path:
        sys.path.append(_p)

import numpy as np

import concourse.bacc as bacc
import concourse.bass as bass
import concourse.tile as tile
from concourse import mybir

F32 = mybir.dt.float32
BF16 = mybir.dt.bfloat16

B, S, D = 2, 1024, 1024
H, DH = 16, 64
HPC = 4                    # heads per core
EC = HPC * DH              # 256 feature cols per core side
NP = 5                     # stored Chebyshev orders 1..5 (f3, g5 halved)
NS = S // 128              # 8 s-tiles
NKD = D // 128             # 8 k-tiles over d for QKV
EPS_DEN = 1e-7
INV_SQRT_D = 1.0 / 8.0     # 1/sqrt(64)
KV = NP * EC               # 1280 kv channels
DEN0 = KV                  # den channels at [1280:1300] of kvt
NDEN = NP * HPC            # 20


def _beta():
    j = np.arange(6, dtype=np.float32)
    alpha = (j + 1.0) ** (-1.5)
    tail = np.flip(np.cumsum(np.flip(alpha)))
    beta = np.concatenate([np.zeros(1, np.float32), tail[1:].astype(np.float32),
                           np.zeros(5, np.float32)])
    return beta / beta.sum()          # (11,); nonzero at 1..5


# beta' with the stored-feature scale folded in (f3 = T3/2, g5 = T5/2)
_BSCALE = np.array([1.0, 1.0, 4.0, 1.0, 4.0], np.float32)


def _mid_bcast(ap, reps, at=1):
    """Insert a stride-0 dim of length `reps` into an AP's free dims."""
    new = list(ap.ap)
    new.insert(at, [0, reps])
    return bass.AP(tensor=ap.tensor, offset=ap.offset, ap=new)


def _build():
    nc = bacc.Bacc("TRN2", target_bir_lowering=False, debug=False, num_devices=8)

    XT = nc.dram_tensor("xt", [D, S], BF16, kind="ExternalInput")
    WQKVT = nc.dram_tensor("wqkvt", [D, 3 * EC], BF16, kind="ExternalInput")
    WOUTT = nc.dram_tensor("woutt", [EC, D], BF16, kind="ExternalInput")
    LTB = nc.dram_tensor("ltb", [6, 128, 128], BF16, kind="ExternalInput")
    BCONST = nc.dram_tensor("bconst", [128, NDEN], F32, kind="ExternalInput")
    PART = nc.dram_tensor("part", [S, D], F32, kind="ExternalOutput")

    OP = mybir.AluOpType
    AX = mybir.AxisListType
    ACT = mybir.ActivationFunctionType

    with tile.TileContext(nc) as tc:
        with (
            nc.allow_low_precision(reason="bf16 feature pipeline by design"),
            tc.tile_pool(name="persist", bufs=1) as pp,
            tc.tile_pool(name="work", bufs=4) as wp,
            tc.tile_pool(name="ps_qkv", bufs=2, space="PSUM") as ps_qkv,
            tc.tile_pool(name="ps_kv", bufs=1, space="PSUM") as ps_kv,
            tc.tile_pool(name="ps_o", bufs=1, space="PSUM") as ps_o,
        ):
            xt = pp.tile([128, NKD, S], BF16)
            wqkvt = pp.tile([128, NKD, 3 * EC], BF16)
            woutt = pp.tile([128, 2, D], BF16)
            ltb = pp.tile([128, 6, 128], BF16)
            bconst = pp.tile([128, NDEN], F32)
            ones1 = pp.tile([1, 128], BF16)

            # interleave weight/x chunk loads so QKV can start early
            for k in range(NKD):
                nc.sync.dma_start(out=wqkvt[:, k, :], in_=WQKVT[128 * k:128 * (k + 1), :])
                nc.scalar.dma_start(out=xt[:, k, :], in_=XT[128 * k:128 * (k + 1), :])
            for k in range(2):
                nc.scalar.dma_start(out=woutt[:, k, :], in_=WOUTT[128 * k:128 * (k + 1), :])
            for p in range(6):
                nc.sync.dma_start(out=ltb[:, p, :], in_=LTB[p])
            nc.sync.dma_start(out=bconst, in_=BCONST.ap())
            nc.vector.memset(ones1, 1.0)

            st = [None] * NS   # per-tile state for the back half
            kvt_prev = None

            def front(i):
                nonlocal kvt_prev
                si = slice(128 * i, 128 * (i + 1))
                first = (i == 0)

                # QKV projection: psum [q 0:256 | k 256:512 | v 512:768]
                qkv = ps_qkv.tile([128, 768], F32, tag="qkv")
                for k in range(NKD):
                    lhs = xt[:, k, si]
                    nc.tensor.matmul(qkv[:, 0:512], lhs, wqkvt[:, k, 0:512],
                                     start=(k == 0), stop=(k == NKD - 1))
                    nc.tensor.matmul(qkv[:, 512:768], lhs, wqkvt[:, k, 512:768],
                                     start=(k == 0), stop=(k == NKD - 1))

                # feature chain; fqk [p, q 0:256 | k 256:512]
                fqk = wp.tile([128, NP, 512], BF16, tag="fqk")
                m2 = wp.tile([128, 512], BF16, tag="m2")
                m4 = wp.tile([128, 512], BF16, tag="m4")
                u5 = wp.tile([128, 512], BF16, tag="u5")
                vt = wp.tile([128, EC], BF16, tag="vt")
                nc.scalar.copy(out=fqk[:, 0, :], in_=qkv[:, 0:512])
                nc.scalar.activation(out=m2, in_=qkv[:, 0:512], func=ACT.Square)
                nc.vector.tensor_copy(out=vt, in_=qkv[:, 512:768])
                nc.vector.tensor_scalar(out=fqk[:, 1, :], in0=m2,
                                        scalar1=2.0, scalar2=-1.0,
                                        op0=OP.mult, op1=OP.add)
                nc.vector.scalar_tensor_tensor(out=fqk[:, 2, :], in0=fqk[:, 1, :],
                                               scalar=-0.5, in1=fqk[:, 0, :],
                                               op0=OP.add, op1=OP.mult)
                nc.scalar.activation(out=m4, in_=fqk[:, 1, :], func=ACT.Square)
                nc.vector.tensor_scalar(out=fqk[:, 3, :], in0=m4,
                                        scalar1=2.0, scalar2=-1.0,
                                        op0=OP.mult, op1=OP.add)
                # T5 = 2x*T4 - T3 -> g5 = x*t4 - f3 = T5/2
                nc.gpsimd.tensor_tensor(out=u5, in0=fqk[:, 0, :],
                                        in1=fqk[:, 3, :], op=OP.mult)
                nc.gpsimd.tensor_tensor(out=fqk[:, 4, :], in0=u5,
                                        in1=fqk[:, 2, :], op=OP.subtract)

                # k-side per-head sums (cumsum den channels)
                sums = wp.tile([128, 2 * NDEN], BF16, tag="sums")
                st[i] = [fqk, sums, None, si]
                nc.vector.tensor_reduce(
                    out=sums[:, NDEN:2 * NDEN].rearrange("a (p h) -> a p h", p=NP),
                    in_=fqk[:, :, 256:512].rearrange("a p (h d) -> a p h d", h=HPC),
                    axis=AX.X, op=OP.add)

                # Tv = Tk * v
                tv = wp.tile([128, NP, EC], BF16, tag="tv")
                nc.gpsimd.tensor_tensor(out=tv, in0=fqk[:, :, 256:512],
                                        in1=_mid_bcast(vt, NP), op=OP.mult)
                st[i].append(tv)

            def mid(i):
                nonlocal kvt_prev
                fqk, sums, _, si, tv = st[i]
                first = (i == 0)

                # causal cumsum: bank0 [p0 p1] bank1 [p2 p3] bank2 [p4|den]
                kv = ps_kv.tile([128, 3, 512], F32, tag="kv")
                for p in range(NP):
                    dst = kv[:, p // 2, 256 * (p % 2):256 * (p % 2) + 256]
                    nc.tensor.matmul(dst, ltb[:, p, :], tv[:, p, :],
                                     start=True, stop=first,
                                     skip_group_check=True)
                dend = kv[:, 2, 256:256 + NDEN]
                nc.tensor.matmul(dend, ltb[:, 5, :], sums[:, NDEN:2 * NDEN],
                                 start=True, stop=first, skip_group_check=True)
                if not first:
                    # bank-wide carry accumulate (row 0 of previous evac)
                    for bk, (lo, w) in enumerate(((0, 512), (512, 512),
                                                  (1024, 276))):
                        nc.tensor.matmul(kv[:, bk, 0:w], ones1,
                                         kvt_prev[0:1, lo:lo + w],
                                         start=False, stop=True,
                                         skip_group_check=True)

                # single evac of kv prefix + den prefix (row 0 = next carry)
                kvt = wp.tile([128, 1300], BF16, tag="kvt")
                nc.scalar.copy(out=kvt,
                               in_=kv.rearrange("a b c -> a (b c)")[:, 0:1300])
                kvt_prev = kvt
                st[i][2] = kvt

            def back(i):
                fqk, sums, kvt, si, _ = st[i]
                st[i] = None

                # q-side per-head sums (for den)
                nc.vector.tensor_reduce(
                    out=sums[:, 0:NDEN].rearrange("a (p h) -> a p h", p=NP),
                    in_=fqk[:, :, 0:256].rearrange("a p (h d) -> a p h d", h=HPC),
                    axis=AX.X, op=OP.add)

                # num: prods then reduce over p
                prods = wp.tile([128, EC, NP], BF16, tag="prods")
                numq = wp.tile([128, EC], F32, tag="numq")
                nc.gpsimd.tensor_tensor(
                    out=prods.rearrange("a e p -> a p e"), in0=fqk[:, :, 0:256],
                    in1=kvt[:, 0:KV].rearrange("a (p e) -> a p e", p=NP),
                    op=OP.mult)
                nc.vector.tensor_reduce(
                    out=numq, in_=prods, axis=AX.X, op=OP.add)

                # den chain
                qsb = wp.tile([128, NDEN], BF16, tag="qsb")
                dpr = wp.tile([128, NDEN], F32, tag="dpr")
                den4 = wp.tile([128, HPC], F32, tag="den4")
                den4e = wp.tile([128, HPC], F32, tag="den4e")
                rden = wp.tile([128, HPC], F32, tag="rden")
                nc.vector.scalar_tensor_tensor(out=qsb, in0=sums[:, 0:NDEN],
                                               scalar=1.0, in1=bconst,
                                               op0=OP.mult, op1=OP.mult)
                nc.vector.scalar_tensor_tensor(out=dpr, in0=qsb, scalar=1.0,
                                               in1=kvt[:, DEN0:DEN0 + NDEN],
                                               op0=OP.mult, op1=OP.mult)
                nc.vector.tensor_reduce(
                    out=den4, in_=dpr.rearrange("a (p h) -> a h p", p=NP),
                    axis=AX.X, op=OP.add)
                nc.vector.tensor_scalar_add(out=den4e, in0=den4, scalar1=EPS_DEN)
                nc.vector.reciprocal(out=rden, in_=den4e)

                # out_h = num * rden (rden broadcast over Dh)
                outh = wp.tile([128, EC], BF16, tag="outh")
                nc.vector.tensor_tensor(
                    out=outh.rearrange("a (h d) -> a h d", h=HPC),
                    in0=numq.rearrange("a (h d) -> a h d", h=HPC),
                    in1=_mid_bcast(rden, DH, at=2), op=OP.mult)

                # transpose via DMA xbar, then two 512-wide projection passes
                outt = wp.tile([128, 2, 128], BF16, tag="outt")
                nc.sync.dma_start_transpose(out=outt, in_=outh)
                for n in range(2):
                    po = ps_o.tile([128, 512], F32, tag="po")
                    for kt in range(2):
                        nc.tensor.matmul(po, outt[:, kt, :],
                                         woutt[:, kt, 512 * n:512 * (n + 1)],
                                         start=(kt == 0), stop=(kt == 1))
                    outfull = wp.tile([128, 512], F32, tag="outfull")
                    nc.scalar.copy(out=outfull, in_=po)
                    nc.sync.dma_start(out=PART[si, 512 * n:512 * (n + 1)],
                                      in_=outfull)

            for i in range(NS + 2):
                if i < NS:
                    front(i)
                if 0 <= i - 1 < NS:
                    mid(i - 1)
                if 0 <= i - 2 < NS:
                    back(i - 2)

    nc.compile()
    return nc


_NC = None


def _get_nc():
    global _NC
    if _NC is None:
        _NC = _build()
    return _NC


def _stage_inputs(x, W_in, W_out):
    import ml_dtypes
    bf = ml_dtypes.bfloat16
    beta = _beta()
    bprime = beta[1:6] * _BSCALE          # (5,)
    tri = np.tril(np.ones((128, 128), np.float32))
    ltb = np.stack([bprime[p] * tri for p in range(5)] + [tri]).astype(bf)
    bconst = np.broadcast_to(np.repeat(bprime, HPC)[None, :],
                             (128, NDEN)).astype(np.float32).copy()
    in_maps = []
    for c in range(8):
        b, hb = divmod(c, 4)
        rs = slice(256 * hb, 256 * (hb + 1))
        wq = W_in[0 * D + 256 * hb:0 * D + 256 * (hb + 1)] * INV_SQRT_D
        wk = W_in[1 * D + 256 * hb:1 * D + 256 * (hb + 1)] * INV_SQRT_D
        wv = W_in[2 * D + 256 * hb:2 * D + 256 * (hb + 1)]
        wqkvt = np.ascontiguousarray(
            np.concatenate([wq, wk, wv], axis=0).T).astype(bf)
        xrev = x[b].T.reshape(D, NS, 128)[:, :, ::-1].reshape(D, S)
        in_maps.append({
            "xt": np.ascontiguousarray(xrev).astype(bf),
            "wqkvt": wqkvt,
            "woutt": np.ascontiguousarray(W_out[:, rs].T).astype(bf),
            "ltb": ltb,
            "bconst": bconst,
        })
    return in_maps


def kernel(x, W_in, W_out):
    from concourse.bass_utils import run_bass_kernel_spmd

    x = np.asarray(x, dtype=np.float32)
    W_in = np.asarray(W_in, dtype=np.float32)
    W_out = np.asarray(W_out, dtype=np.float32)
    nc = _get_nc()
    in_maps = _stage_inputs(x, W_in, W_out)
    res = run_bass_kernel_spmd(nc, in_maps, core_ids=list(range(8)))
    out = np.zeros((B, S, D), dtype=np.float32)
    for c in range(8):
        part = res.results[c]["part"].reshape(NS, 128, D)[:, ::-1, :].reshape(S, D)
        out[c // 4] += part
    return out


# revision 11
# speedup vs baseline: 1.0175x; 1.0129x over previous
"""Trainium2 Bass kernel for CollapsedPBFA (collapsed Chebyshev linear attention).

Full-input contract: kernel(x, W_in, W_out) -> (B, S, D) float32.

Sharding: B x H = 2 x 16 = 32 (batch, head) pairs; each of the 8 cores owns
one batch element's 4-head block (cores 0-3 -> b=0, cores 4-7 -> b=1).
QKV projection is column-parallel per head block; the output projection is
row-parallel and the host sums the per-core partials per batch element.

Structure (per 128-row s-tile, software-pipelined front/back issue order):
  front(i): QKV matmuls -> feature chain (f3 = T3/2, g5 = T5/2 stored, the
    2^a factors folded into the beta' cumsum stationaries / den consts) ->
    per-head feature sums -> Tv -> triangular-matmul causal cumsum with per-p
    carry matmuls -> single scalar-engine evac of the 3-bank psum prefix.
  back(i): prods -> p-reduction -> den chain -> out_h -> DMA-xbar transpose
    -> output projection (two 512-wide passes) -> DMA out.
"""

import sys

for _p in ("/opt/trn_rl_repo", "/root/.axon_site/_ro/trn_rl_repo"):
    if _p not in sys.path:
        sys.path.append(_p)

import numpy as np

import concourse.bacc as bacc
import concourse.bass as bass
import concourse.tile as tile
from concourse import mybir

F32 = mybir.dt.float32
BF16 = mybir.dt.bfloat16

B, S, D = 2, 1024, 1024
H, DH = 16, 64
HPC = 4                    # heads per core
EC = HPC * DH              # 256 feature cols per core side
NP = 5                     # stored Chebyshev orders 1..5 (f3, g5 halved)
NS = S // 128              # 8 s-tiles
NKD = D // 128             # 8 k-tiles over d for QKV
EPS_DEN = 1e-7
INV_SQRT_D = 1.0 / 8.0     # 1/sqrt(64)
KV = NP * EC               # 1280 kv channels
DEN0 = KV                  # den channels at [1280:1300] of kvt
NDEN = NP * HPC            # 20


def _beta():
    j = np.arange(6, dtype=np.float32)
    alpha = (j + 1.0) ** (-1.5)
    tail = np.flip(np.cumsum(np.flip(alpha)))
    beta = np.concatenate([np.zeros(1, np.float32), tail[1:].astype(np.float32),
                           np.zeros(5, np.float32)])
    return beta / beta.sum()          # (11,); nonzero at 1..5


# beta' with the stored-feature scale folded in (f3 = T3/2, g5 = T5/2)
_BSCALE = np.array([1.0, 1.0, 4.0, 1.0, 4.0], np.float32)


def _mid_bcast(ap, reps, at=1):
    """Insert a stride-0 dim of length `reps` into an AP's free dims."""
    new = list(ap.ap)
    new.insert(at, [0, reps])
    return bass.AP(tensor=ap.tensor, offset=ap.offset, ap=new)


def _build():
    nc = bacc.Bacc("TRN2", target_bir_lowering=False, debug=False, num_devices=8)

    XT = nc.dram_tensor("xt", [D, S], BF16, kind="ExternalInput")
    WQKVT = nc.dram_tensor("wqkvt", [D, 3 * EC], BF16, kind="ExternalInput")
    WOUTT = nc.dram_tensor("woutt", [EC, D], BF16, kind="ExternalInput")
    LTB = nc.dram_tensor("ltb", [6, 128, 128], BF16, kind="ExternalInput")
    BCONST = nc.dram_tensor("bconst", [128, NDEN], F32, kind="ExternalInput")
    PART = nc.dram_tensor("part", [S, D], F32, kind="ExternalOutput")

    OP = mybir.AluOpType
    AX = mybir.AxisListType
    ACT = mybir.ActivationFunctionType

    with tile.TileContext(nc) as tc:
        with (
            nc.allow_low_precision(reason="bf16 feature pipeline by design"),
            tc.tile_pool(name="persist", bufs=1) as pp,
            tc.tile_pool(name="work", bufs=4) as wp,
            tc.tile_pool(name="ps_qkv", bufs=2, space="PSUM") as ps_qkv,
            tc.tile_pool(name="ps_kv", bufs=1, space="PSUM") as ps_kv,
            tc.tile_pool(name="ps_o", bufs=1, space="PSUM") as ps_o,
        ):
            xt = pp.tile([128, NKD, S], BF16)
            wqkvt = pp.tile([128, NKD, 3 * EC], BF16)
            woutt = pp.tile([128, 2, D], BF16)
            ltb = pp.tile([128, 6, 128], BF16)
            bconst = pp.tile([128, NDEN], F32)
            ones1 = pp.tile([1, 128], BF16)

            # interleave weight/x chunk loads so QKV can start early
            for k in range(NKD):
                nc.sync.dma_start(out=wqkvt[:, k, :], in_=WQKVT[128 * k:128 * (k + 1), :])
                nc.scalar.dma_start(out=xt[:, k, :], in_=XT[128 * k:128 * (k + 1), :])
            for k in range(2):
                nc.scalar.dma_start(out=woutt[:, k, :], in_=WOUTT[128 * k:128 * (k + 1), :])
            for p in range(6):
                nc.sync.dma_start(out=ltb[:, p, :], in_=LTB[p])
            nc.sync.dma_start(out=bconst, in_=BCONST.ap())
            nc.vector.memset(ones1, 1.0)

            st = [None] * NS   # per-tile state for the back half
            kvt_prev = None

            def front(i):
                nonlocal kvt_prev
                si = slice(128 * i, 128 * (i + 1))
                first = (i == 0)

                # QKV projection: psum [q 0:256 | k 256:512 | v 512:768]
                qkv = ps_qkv.tile([128, 768], F32, tag="qkv")
                for k in range(NKD):
                    lhs = xt[:, k, si]
                    nc.tensor.matmul(qkv[:, 0:512], lhs, wqkvt[:, k, 0:512],
                                     start=(k == 0), stop=(k == NKD - 1))
                    nc.tensor.matmul(qkv[:, 512:768], lhs, wqkvt[:, k, 512:768],
                                     start=(k == 0), stop=(k == NKD - 1))

                # feature chain; fqk [p, q 0:256 | k 256:512]
                fqk = wp.tile([128, NP, 512], BF16, tag="fqk")
                m2 = wp.tile([128, 512], BF16, tag="m2")
                m4 = wp.tile([128, 512], BF16, tag="m4")
                u5 = wp.tile([128, 512], BF16, tag="u5")
                vt = wp.tile([128, EC], BF16, tag="vt")
                nc.scalar.copy(out=fqk[:, 0, :], in_=qkv[:, 0:512])
                nc.scalar.activation(out=m2, in_=qkv[:, 0:512], func=ACT.Square)
                nc.vector.tensor_copy(out=vt, in_=qkv[:, 512:768])
                nc.vector.tensor_scalar(out=fqk[:, 1, :], in0=m2,
                                        scalar1=2.0, scalar2=-1.0,
                                        op0=OP.mult, op1=OP.add)
                nc.vector.scalar_tensor_tensor(out=fqk[:, 2, :], in0=fqk[:, 1, :],
                                               scalar=-0.5, in1=fqk[:, 0, :],
                                               op0=OP.add, op1=OP.mult)
                nc.scalar.activation(out=m4, in_=fqk[:, 1, :], func=ACT.Square)
                nc.vector.tensor_scalar(out=fqk[:, 3, :], in0=m4,
                                        scalar1=2.0, scalar2=-1.0,
                                        op0=OP.mult, op1=OP.add)
                # T5 = 2x*T4 - T3 -> g5 = x*t4 - f3 = T5/2
                nc.gpsimd.tensor_tensor(out=u5, in0=fqk[:, 0, :],
                                        in1=fqk[:, 3, :], op=OP.mult)
                nc.gpsimd.tensor_tensor(out=fqk[:, 4, :], in0=u5,
                                        in1=fqk[:, 2, :], op=OP.subtract)

                # k-side per-head sums (cumsum den channels)
                sums = wp.tile([128, 2 * NDEN], BF16, tag="sums")
                st[i] = [fqk, sums, None, si]
                nc.vector.tensor_reduce(
                    out=sums[:, NDEN:2 * NDEN].rearrange("a (p h) -> a p h", p=NP),
                    in_=fqk[:, :, 256:512].rearrange("a p (h d) -> a p h d", h=HPC),
                    axis=AX.X, op=OP.add)

                # Tv = Tk * v
                tv = wp.tile([128, NP, EC], BF16, tag="tv")
                nc.gpsimd.tensor_tensor(out=tv, in0=fqk[:, :, 256:512],
                                        in1=_mid_bcast(vt, NP), op=OP.mult)
                st[i].append(tv)

            def mid(i):
                nonlocal kvt_prev
                fqk, sums, _, si, tv = st[i]
                first = (i == 0)

                # causal cumsum: bank0 [p0 p1] bank1 [p2 p3] bank2 [p4|den]
                kv = ps_kv.tile([128, 3, 512], F32, tag="kv")
                for p in range(NP):
                    dst = kv[:, p // 2, 256 * (p % 2):256 * (p % 2) + 256]
                    nc.tensor.matmul(dst, ltb[:, p, :], tv[:, p, :],
                                     start=True, stop=first,
                                     skip_group_check=True)
                dend = kv[:, 2, 256:256 + NDEN]
                nc.tensor.matmul(dend, ltb[:, 5, :], sums[:, NDEN:2 * NDEN],
                                 start=True, stop=first, skip_group_check=True)
                if not first:
                    # bank-wide carry accumulate (row 0 of previous evac)
                    for bk, (lo, w) in enumerate(((0, 512), (512, 512),
                                                  (1024, 276))):
                        nc.tensor.matmul(kv[:, bk, 0:w], ones1,
                                         kvt_prev[0:1, lo:lo + w],
                                         start=False, stop=True,
                                         skip_group_check=True)

                # single evac of kv prefix + den prefix (row 0 = next carry)
                kvt = wp.tile([128, 1300], BF16, tag="kvt")
                nc.scalar.copy(out=kvt,
                               in_=kv.rearrange("a b c -> a (b c)")[:, 0:1300])
                kvt_prev = kvt
                st[i][2] = kvt

            def back(i):
                fqk, sums, kvt, si, _ = st[i]
                st[i] = None

                # q-side per-head sums (for den)
                nc.vector.tensor_reduce(
                    out=sums[:, 0:NDEN].rearrange("a (p h) -> a p h", p=NP),
                    in_=fqk[:, :, 0:256].rearrange("a p (h d) -> a p h d", h=HPC),
                    axis=AX.X, op=OP.add)

                # num: prods then reduce over p
                prods = wp.tile([128, NP, EC], BF16, tag="prods")
                numq = wp.tile([128, EC], F32, tag="numq")
                nc.gpsimd.tensor_tensor(
                    out=prods, in0=fqk[:, :, 0:256],
                    in1=kvt[:, 0:KV].rearrange("a (p e) -> a p e", p=NP),
                    op=OP.mult)
                nc.vector.tensor_reduce(
                    out=numq, in_=prods.rearrange("a p e -> a e p"),
                    axis=AX.X, op=OP.add)

                # den chain
                qsb = wp.tile([128, NDEN], BF16, tag="qsb")
                dpr = wp.tile([128, NDEN], F32, tag="dpr")
                den4 = wp.tile([128, HPC], F32, tag="den4")
                den4e = wp.tile([128, HPC], F32, tag="den4e")
                rden = wp.tile([128, HPC], F32, tag="rden")
                nc.gpsimd.tensor_tensor(out=qsb, in0=sums[:, 0:NDEN],
                                        in1=bconst, op=OP.mult)
                nc.vector.scalar_tensor_tensor(out=dpr, in0=qsb, scalar=1.0,
                                               in1=kvt[:, DEN0:DEN0 + NDEN],
                                               op0=OP.mult, op1=OP.mult)
                nc.vector.tensor_reduce(
                    out=den4, in_=dpr.rearrange("a (p h) -> a h p", p=NP),
                    axis=AX.X, op=OP.add)
                nc.vector.tensor_scalar_add(out=den4e, in0=den4, scalar1=EPS_DEN)
                nc.vector.reciprocal(out=rden, in_=den4e)

                # out_h = num * rden (rden broadcast over Dh)
                outh = wp.tile([128, EC], BF16, tag="outh")
                nc.gpsimd.tensor_tensor(
                    out=outh.rearrange("a (h d) -> a h d", h=HPC),
                    in0=numq.rearrange("a (h d) -> a h d", h=HPC),
                    in1=_mid_bcast(rden, DH, at=2), op=OP.mult)

                # transpose via DMA xbar, then two 512-wide projection passes
                outt = wp.tile([128, 2, 128], BF16, tag="outt")
                nc.sync.dma_start_transpose(out=outt, in_=outh)
                for n in range(2):
                    po = ps_o.tile([128, 512], F32, tag="po")
                    for kt in range(2):
                        nc.tensor.matmul(po, outt[:, kt, :],
                                         woutt[:, kt, 512 * n:512 * (n + 1)],
                                         start=(kt == 0), stop=(kt == 1))
                    outfull = wp.tile([128, 512], F32, tag="outfull")
                    nc.scalar.copy(out=outfull, in_=po)
                    nc.sync.dma_start(out=PART[si, 512 * n:512 * (n + 1)],
                                      in_=outfull)

            for i in range(NS + 2):
                if i < NS:
                    front(i)
                if 0 <= i - 1 < NS:
                    mid(i - 1)
                if 0 <= i - 2 < NS:
                    back(i - 2)

    nc.compile()
    return nc


_NC = None


def _get_nc():
    global _NC
    if _NC is None:
        _NC = _build()
    return _NC


def _stage_inputs(x, W_in, W_out):
    import ml_dtypes
    bf = ml_dtypes.bfloat16
    beta = _beta()
    bprime = beta[1:6] * _BSCALE          # (5,)
    tri = np.tril(np.ones((128, 128), np.float32))
    ltb = np.stack([bprime[p] * tri for p in range(5)] + [tri]).astype(bf)
    bconst = np.broadcast_to(np.repeat(bprime, HPC)[None, :],
                             (128, NDEN)).astype(np.float32).copy()
    in_maps = []
    for c in range(8):
        b, hb = divmod(c, 4)
        rs = slice(256 * hb, 256 * (hb + 1))
        wq = W_in[0 * D + 256 * hb:0 * D + 256 * (hb + 1)] * INV_SQRT_D
        wk = W_in[1 * D + 256 * hb:1 * D + 256 * (hb + 1)] * INV_SQRT_D
        wv = W_in[2 * D + 256 * hb:2 * D + 256 * (hb + 1)]
        wqkvt = np.ascontiguousarray(
            np.concatenate([wq, wk, wv], axis=0).T).astype(bf)
        xrev = x[b].T.reshape(D, NS, 128)[:, :, ::-1].reshape(D, S)
        in_maps.append({
            "xt": np.ascontiguousarray(xrev).astype(bf),
            "wqkvt": wqkvt,
            "woutt": np.ascontiguousarray(W_out[:, rs].T).astype(bf),
            "ltb": ltb,
            "bconst": bconst,
        })
    return in_maps


def kernel(x, W_in, W_out):
    from concourse.bass_utils import run_bass_kernel_spmd

    x = np.asarray(x, dtype=np.float32)
    W_in = np.asarray(W_in, dtype=np.float32)
    W_out = np.asarray(W_out, dtype=np.float32)
    nc = _get_nc()
    in_maps = _stage_inputs(x, W_in, W_out)
    res = run_bass_kernel_spmd(nc, in_maps, core_ids=list(range(8)))
    out = np.zeros((B, S, D), dtype=np.float32)
    for c in range(8):
        part = res.results[c]["part"].reshape(NS, 128, D)[:, ::-1, :].reshape(S, D)
        out[c // 4] += part
    return out


# revision 12
# speedup vs baseline: 1.0585x; 1.0403x over previous
"""Trainium2 Bass kernel for CollapsedPBFA (collapsed Chebyshev linear attention).

Full-input contract: kernel(x, W_in, W_out) -> (B, S, D) float32.

Sharding: B x H = 2 x 16 = 32 (batch, head) pairs; each of the 8 cores owns
one batch element's 4-head block (cores 0-3 -> b=0, cores 4-7 -> b=1).
QKV projection is column-parallel per head block; the output projection is
row-parallel and the host sums the per-core partials per batch element.

Structure (per 128-row s-tile, software-pipelined front/back issue order):
  front(i): QKV matmuls -> feature chain (f3 = T3/2, g5 = T5/2 stored, the
    2^a factors folded into the beta' cumsum stationaries / den consts) ->
    per-head feature sums -> Tv -> triangular-matmul causal cumsum with per-p
    carry matmuls -> single scalar-engine evac of the 3-bank psum prefix.
  back(i): prods -> p-reduction -> den chain -> out_h -> DMA-xbar transpose
    -> output projection (two 512-wide passes) -> DMA out.
"""

import sys

for _p in ("/opt/trn_rl_repo", "/root/.axon_site/_ro/trn_rl_repo"):
    if _p not in sys.path:
        sys.path.append(_p)

import numpy as np

import concourse.bacc as bacc
import concourse.bass as bass
import concourse.tile as tile
from concourse import mybir

F32 = mybir.dt.float32
BF16 = mybir.dt.bfloat16

B, S, D = 2, 1024, 1024
H, DH = 16, 64
HPC = 4                    # heads per core
EC = HPC * DH              # 256 feature cols per core side
NP = 5                     # stored Chebyshev orders 1..5 (f3, g5 halved)
NS = S // 128              # 8 s-tiles
NKD = D // 128             # 8 k-tiles over d for QKV
EPS_DEN = 1e-7
INV_SQRT_D = 1.0 / 8.0     # 1/sqrt(64)
KV = NP * EC               # 1280 kv channels
DEN0 = KV                  # den channels at [1280:1300] of kvt
NDEN = NP * HPC            # 20


def _beta():
    j = np.arange(6, dtype=np.float32)
    alpha = (j + 1.0) ** (-1.5)
    tail = np.flip(np.cumsum(np.flip(alpha)))
    beta = np.concatenate([np.zeros(1, np.float32), tail[1:].astype(np.float32),
                           np.zeros(5, np.float32)])
    return beta / beta.sum()          # (11,); nonzero at 1..5


# beta' with the stored-feature scale folded in (f3 = T3/2, g5 = T5/2)
_BSCALE = np.array([1.0, 1.0, 4.0, 1.0, 4.0], np.float32)


def _mid_bcast(ap, reps, at=1):
    """Insert a stride-0 dim of length `reps` into an AP's free dims."""
    new = list(ap.ap)
    new.insert(at, [0, reps])
    return bass.AP(tensor=ap.tensor, offset=ap.offset, ap=new)


def _build():
    nc = bacc.Bacc("TRN2", target_bir_lowering=False, debug=False, num_devices=8)

    XT = nc.dram_tensor("xt", [D, S], BF16, kind="ExternalInput")
    WQKVT = nc.dram_tensor("wqkvt", [D, 3 * EC], BF16, kind="ExternalInput")
    WOUTT = nc.dram_tensor("woutt", [EC, D], BF16, kind="ExternalInput")
    LTB = nc.dram_tensor("ltb", [6, 128, 128], BF16, kind="ExternalInput")
    BCONST = nc.dram_tensor("bconst", [128, NDEN], F32, kind="ExternalInput")
    PART = nc.dram_tensor("part", [S, D], F32, kind="ExternalOutput")

    OP = mybir.AluOpType
    AX = mybir.AxisListType
    ACT = mybir.ActivationFunctionType

    with tile.TileContext(nc) as tc:
        with (
            nc.allow_low_precision(reason="bf16 feature pipeline by design"),
            tc.tile_pool(name="persist", bufs=1) as pp,
            tc.tile_pool(name="work", bufs=4) as wp,
            tc.tile_pool(name="ps_qkv", bufs=2, space="PSUM") as ps_qkv,
            tc.tile_pool(name="ps_kv", bufs=1, space="PSUM") as ps_kv,
            tc.tile_pool(name="ps_o", bufs=1, space="PSUM") as ps_o,
        ):
            xt = pp.tile([128, NKD, S], BF16)
            wqkvt = pp.tile([128, NKD, 3 * EC], BF16)
            woutt = pp.tile([128, 2, D], BF16)
            ltb = pp.tile([128, 6, 128], BF16)
            bconst = pp.tile([128, NDEN], F32)
            ones1 = pp.tile([1, 128], BF16)

            # interleave weight/x chunk loads so QKV can start early
            for k in range(NKD):
                nc.sync.dma_start(out=wqkvt[:, k, :], in_=WQKVT[128 * k:128 * (k + 1), :])
                nc.scalar.dma_start(out=xt[:, k, :], in_=XT[128 * k:128 * (k + 1), :])
            for k in range(2):
                nc.scalar.dma_start(out=woutt[:, k, :], in_=WOUTT[128 * k:128 * (k + 1), :])
            for p in range(6):
                nc.sync.dma_start(out=ltb[:, p, :], in_=LTB[p])
            nc.sync.dma_start(out=bconst, in_=BCONST.ap())
            nc.vector.memset(ones1, 1.0)

            st = [None] * NS   # per-tile state for the back half
            kvt_prev = None

            def front(i):
                nonlocal kvt_prev
                si = slice(128 * i, 128 * (i + 1))
                first = (i == 0)

                # QKV projection: psum [q 0:256 | k 256:512 | v 512:768]
                qkv = ps_qkv.tile([128, 768], F32, tag="qkv")
                for k in range(NKD):
                    lhs = xt[:, k, si]
                    nc.tensor.matmul(qkv[:, 0:512], lhs, wqkvt[:, k, 0:512],
                                     start=(k == 0), stop=(k == NKD - 1))
                    nc.tensor.matmul(qkv[:, 512:768], lhs, wqkvt[:, k, 512:768],
                                     start=(k == 0), stop=(k == NKD - 1))

                # feature chain; fqk [p, q 0:256 | k 256:512]
                fqk = wp.tile([128, NP, 512], BF16, tag="fqk")
                m2 = wp.tile([128, 512], BF16, tag="m2")
                m4 = wp.tile([128, 512], BF16, tag="m4")
                u5 = wp.tile([128, 512], BF16, tag="u5")
                vt = wp.tile([128, EC], BF16, tag="vt")
                nc.scalar.copy(out=fqk[:, 0, :], in_=qkv[:, 0:512])
                nc.scalar.activation(out=m2, in_=qkv[:, 0:512], func=ACT.Square)
                nc.vector.tensor_copy(out=vt, in_=qkv[:, 512:768])
                nc.vector.tensor_scalar(out=fqk[:, 1, :], in0=m2,
                                        scalar1=2.0, scalar2=-1.0,
                                        op0=OP.mult, op1=OP.add)
                nc.vector.scalar_tensor_tensor(out=fqk[:, 2, :], in0=fqk[:, 1, :],
                                               scalar=-0.5, in1=fqk[:, 0, :],
                                               op0=OP.add, op1=OP.mult)
                nc.scalar.activation(out=m4, in_=fqk[:, 1, :], func=ACT.Square)
                nc.vector.tensor_scalar(out=fqk[:, 3, :], in0=m4,
                                        scalar1=2.0, scalar2=-1.0,
                                        op0=OP.mult, op1=OP.add)
                # T5 = 2x*T4 - T3 -> g5 = x*t4 - f3 = T5/2
                nc.gpsimd.tensor_tensor(out=u5, in0=fqk[:, 0, :],
                                        in1=fqk[:, 3, :], op=OP.mult)
                nc.gpsimd.tensor_tensor(out=fqk[:, 4, :], in0=u5,
                                        in1=fqk[:, 2, :], op=OP.subtract)

                # k-side per-head sums (cumsum den channels)
                sums = wp.tile([128, 2 * NDEN], BF16, tag="sums")
                st[i] = [fqk, sums, None, si]
                nc.vector.tensor_reduce(
                    out=sums[:, NDEN:2 * NDEN].rearrange("a (p h) -> a p h", p=NP),
                    in_=fqk[:, :, 256:512].rearrange("a p (h d) -> a p h d", h=HPC),
                    axis=AX.X, op=OP.add)

                # Tv = Tk * v
                tv = wp.tile([128, NP, EC], BF16, tag="tv")
                nc.gpsimd.tensor_tensor(out=tv, in0=fqk[:, :, 256:512],
                                        in1=_mid_bcast(vt, NP), op=OP.mult)
                st[i].append(tv)

            def mid(i):
                nonlocal kvt_prev
                fqk, sums, _, si, tv = st[i]
                first = (i == 0)

                # causal cumsum: bank0 [p0 p1] bank1 [p2 p3] bank2 [p4|den]
                kv = ps_kv.tile([128, 3, 512], F32, tag="kv")
                for p in range(NP):
                    dst = kv[:, p // 2, 256 * (p % 2):256 * (p % 2) + 256]
                    nc.tensor.matmul(dst, ltb[:, p, :], tv[:, p, :],
                                     start=True, stop=first,
                                     skip_group_check=True)
                dend = kv[:, 2, 256:256 + NDEN]
                nc.tensor.matmul(dend, ltb[:, 5, :], sums[:, NDEN:2 * NDEN],
                                 start=True, stop=first, skip_group_check=True)
                if not first:
                    # bank-wide carry accumulate (row 0 of previous evac)
                    for bk, (lo, w) in enumerate(((0, 512), (512, 512),
                                                  (1024, 276))):
                        nc.tensor.matmul(kv[:, bk, 0:w], ones1,
                                         kvt_prev[0:1, lo:lo + w],
                                         start=False, stop=True,
                                         skip_group_check=True)

                # single evac of kv prefix + den prefix (row 0 = next carry)
                kvt = wp.tile([128, 1300], BF16, tag="kvt")
                nc.scalar.copy(out=kvt,
                               in_=kv.rearrange("a b c -> a (b c)")[:, 0:1300])
                kvt_prev = kvt
                st[i][2] = kvt

            def back(i):
                fqk, sums, kvt, si, _ = st[i]
                st[i] = None

                # q-side per-head sums (for den)
                nc.vector.tensor_reduce(
                    out=sums[:, 0:NDEN].rearrange("a (p h) -> a p h", p=NP),
                    in_=fqk[:, :, 0:256].rearrange("a p (h d) -> a p h d", h=HPC),
                    axis=AX.X, op=OP.add)

                # num: prods then reduce over p
                prods = wp.tile([128, NP, EC], BF16, tag="prods")
                numq = wp.tile([128, EC], F32, tag="numq")
                nc.gpsimd.tensor_tensor(
                    out=prods, in0=fqk[:, :, 0:256],
                    in1=kvt[:, 0:KV].rearrange("a (p e) -> a p e", p=NP),
                    op=OP.mult)
                nc.vector.tensor_reduce(
                    out=numq, in_=prods.rearrange("a p e -> a e p"),
                    axis=AX.X, op=OP.add)

                # den chain
                qsb = wp.tile([128, NDEN], BF16, tag="qsb")
                dpr = wp.tile([128, NDEN], F32, tag="dpr")
                den4 = wp.tile([128, HPC], F32, tag="den4")
                den4e = wp.tile([128, HPC], F32, tag="den4e")
                rden = wp.tile([128, HPC], F32, tag="rden")
                nc.vector.scalar_tensor_tensor(out=qsb, in0=sums[:, 0:NDEN],
                                               scalar=1.0, in1=bconst,
                                               op0=OP.mult, op1=OP.mult)
                nc.vector.scalar_tensor_tensor(out=dpr, in0=qsb, scalar=1.0,
                                               in1=kvt[:, DEN0:DEN0 + NDEN],
                                               op0=OP.mult, op1=OP.mult)
                nc.vector.tensor_reduce(
                    out=den4, in_=dpr.rearrange("a (p h) -> a h p", p=NP),
                    axis=AX.X, op=OP.add)
                nc.vector.tensor_scalar_add(out=den4e, in0=den4, scalar1=EPS_DEN)
                nc.vector.reciprocal(out=rden, in_=den4e)

                # out_h = num * rden (rden broadcast over Dh)
                outh = wp.tile([128, EC], BF16, tag="outh")
                nc.vector.tensor_tensor(
                    out=outh.rearrange("a (h d) -> a h d", h=HPC),
                    in0=numq.rearrange("a (h d) -> a h d", h=HPC),
                    in1=_mid_bcast(rden, DH, at=2), op=OP.mult)

                # transpose via DMA xbar, then two 512-wide projection passes
                outt = wp.tile([128, 2, 128], BF16, tag="outt")
                nc.sync.dma_start_transpose(out=outt, in_=outh)
                for n in range(2):
                    po = ps_o.tile([128, 512], F32, tag="po")
                    for kt in range(2):
                        nc.tensor.matmul(po, outt[:, kt, :],
                                         woutt[:, kt, 512 * n:512 * (n + 1)],
                                         start=(kt == 0), stop=(kt == 1))
                    outfull = wp.tile([128, 512], F32, tag="outfull")
                    nc.scalar.copy(out=outfull, in_=po)
                    nc.sync.dma_start(out=PART[si, 512 * n:512 * (n + 1)],
                                      in_=outfull)

            for i in range(NS + 2):
                if i < NS:
                    front(i)
                if 0 <= i - 1 < NS:
                    mid(i - 1)
                if 0 <= i - 2 < NS:
                    back(i - 2)

    nc.compile()
    return nc


_NC = None


def _get_nc():
    global _NC
    if _NC is None:
        _NC = _build()
    return _NC


def _stage_inputs(x, W_in, W_out):
    import ml_dtypes
    bf = ml_dtypes.bfloat16
    beta = _beta()
    bprime = beta[1:6] * _BSCALE          # (5,)
    tri = np.tril(np.ones((128, 128), np.float32))
    ltb = np.stack([bprime[p] * tri for p in range(5)] + [tri]).astype(bf)
    bconst = np.broadcast_to(np.repeat(bprime, HPC)[None, :],
                             (128, NDEN)).astype(np.float32).copy()
    in_maps = []
    for c in range(8):
        b, hb = divmod(c, 4)
        rs = slice(256 * hb, 256 * (hb + 1))
        wq = W_in[0 * D + 256 * hb:0 * D + 256 * (hb + 1)] * INV_SQRT_D
        wk = W_in[1 * D + 256 * hb:1 * D + 256 * (hb + 1)] * INV_SQRT_D
        wv = W_in[2 * D + 256 * hb:2 * D + 256 * (hb + 1)]
        wqkvt = np.ascontiguousarray(
            np.concatenate([wq, wk, wv], axis=0).T).astype(bf)
        xrev = x[b].T.reshape(D, NS, 128)[:, :, ::-1].reshape(D, S)
        in_maps.append({
            "xt": np.ascontiguousarray(xrev).astype(bf),
            "wqkvt": wqkvt,
            "woutt": np.ascontiguousarray(W_out[:, rs].T).astype(bf),
            "ltb": ltb,
            "bconst": bconst,
        })
    return in_maps


def kernel(x, W_in, W_out):
    from concourse.bass_utils import run_bass_kernel_spmd

    x = np.asarray(x, dtype=np.float32)
    W_in = np.asarray(W_in, dtype=np.float32)
    W_out = np.asarray(W_out, dtype=np.float32)
    nc = _get_nc()
    in_maps = _stage_inputs(x, W_in, W_out)
    res = run_bass_kernel_spmd(nc, in_maps, core_ids=list(range(8)))
    out = np.zeros((B, S, D), dtype=np.float32)
    for c in range(8):
        part = res.results[c]["part"].reshape(NS, 128, D)[:, ::-1, :].reshape(S, D)
        out[c // 4] += part
    return out


# revision 16
# speedup vs baseline: 1.0650x; 1.0061x over previous
"""Trainium2 Bass kernel for CollapsedPBFA (collapsed Chebyshev linear attention).

Full-input contract: kernel(x, W_in, W_out) -> (B, S, D) float32.

Sharding: B x H = 2 x 16 = 32 (batch, head) pairs; each of the 8 cores owns
one batch element's 4-head block (cores 0-3 -> b=0, cores 4-7 -> b=1).
QKV projection is column-parallel per head block; the output projection is
row-parallel and the host sums the per-core partials per batch element.

Structure (per 128-row s-tile, software-pipelined front/back issue order):
  front(i): QKV matmuls -> feature chain (f3 = T3/2, g5 = T5/2 stored, the
    2^a factors folded into the beta' cumsum stationaries / den consts) ->
    per-head feature sums -> Tv -> triangular-matmul causal cumsum with per-p
    carry matmuls -> single scalar-engine evac of the 3-bank psum prefix.
  back(i): prods -> p-reduction -> den chain -> out_h -> DMA-xbar transpose
    -> output projection (two 512-wide passes) -> DMA out.
"""

import sys

for _p in ("/opt/trn_rl_repo", "/root/.axon_site/_ro/trn_rl_repo"):
    if _p not in sys.path:
        sys.path.append(_p)

import numpy as np

import concourse.bacc as bacc
import concourse.bass as bass
import concourse.tile as tile
from concourse import mybir

F32 = mybir.dt.float32
BF16 = mybir.dt.bfloat16

B, S, D = 2, 1024, 1024
H, DH = 16, 64
HPC = 4                    # heads per core
EC = HPC * DH              # 256 feature cols per core side
NP = 5                     # stored Chebyshev orders 1..5 (f3, g5 halved)
NS = S // 128              # 8 s-tiles
NKD = D // 128             # 8 k-tiles over d for QKV
EPS_DEN = 1e-7
INV_SQRT_D = 1.0 / 8.0     # 1/sqrt(64)
KV = NP * EC               # 1280 kv channels
DEN0 = KV                  # den channels at [1280:1300] of kvt
NDEN = NP * HPC            # 20


def _beta():
    j = np.arange(6, dtype=np.float32)
    alpha = (j + 1.0) ** (-1.5)
    tail = np.flip(np.cumsum(np.flip(alpha)))
    beta = np.concatenate([np.zeros(1, np.float32), tail[1:].astype(np.float32),
                           np.zeros(5, np.float32)])
    return beta / beta.sum()          # (11,); nonzero at 1..5


# beta' with the stored-feature scale folded in (f3 = T3/2, g5 = T5/2)
_BSCALE = np.array([1.0, 1.0, 4.0, 1.0, 4.0], np.float32)


def _mid_bcast(ap, reps, at=1):
    """Insert a stride-0 dim of length `reps` into an AP's free dims."""
    new = list(ap.ap)
    new.insert(at, [0, reps])
    return bass.AP(tensor=ap.tensor, offset=ap.offset, ap=new)


def _build():
    nc = bacc.Bacc("TRN2", target_bir_lowering=False, debug=False, num_devices=8)

    XT = nc.dram_tensor("xt", [D, S], BF16, kind="ExternalInput")
    WQKVT = nc.dram_tensor("wqkvt", [D, 3 * EC + 8], BF16, kind="ExternalInput")
    WOUTT = nc.dram_tensor("woutt", [EC, D], BF16, kind="ExternalInput")
    LTB = nc.dram_tensor("ltb", [6, 128, 128], BF16, kind="ExternalInput")
    BCONST = nc.dram_tensor("bconst", [128, NDEN], F32, kind="ExternalInput")
    PART = nc.dram_tensor("part", [S, D], F32, kind="ExternalOutput")

    OP = mybir.AluOpType
    AX = mybir.AxisListType
    ACT = mybir.ActivationFunctionType

    with tile.TileContext(nc) as tc:
        with (
            nc.allow_low_precision(reason="bf16 feature pipeline by design"),
            tc.tile_pool(name="persist", bufs=1) as pp,
            tc.tile_pool(name="work", bufs=4) as wp,
            tc.tile_pool(name="ps_qkv", bufs=2, space="PSUM") as ps_qkv,
            tc.tile_pool(name="ps_kv", bufs=1, space="PSUM") as ps_kv,
            tc.tile_pool(name="ps_o", bufs=1, space="PSUM") as ps_o,
        ):
            xt = [pp.tile([128, S], BF16, name=f"xt{k}") for k in range(NKD)]
            wqkvt = [pp.tile([128, 3 * EC + 8], BF16, name=f"wqkvt{k}")
                     for k in range(NKD)]
            woutt = pp.tile([128, 2, D], BF16)
            ltb = pp.tile([128, 6, 128], BF16)
            bconst = pp.tile([128, NDEN], F32)
            ones1 = pp.tile([1, 128], BF16)
            dpr8 = pp.tile([128, NS, NDEN + HPC], F32)

            # interleave weight/x chunk loads so QKV can start early
            for k in range(NKD):
                nc.sync.dma_start(out=wqkvt[k], in_=WQKVT[128 * k:128 * (k + 1), :])
                nc.scalar.dma_start(out=xt[k], in_=XT[128 * k:128 * (k + 1), :])
            for k in range(2):
                nc.scalar.dma_start(out=woutt[:, k, :], in_=WOUTT[128 * k:128 * (k + 1), :])
            for p in range(6):
                nc.sync.dma_start(out=ltb[:, p, :], in_=LTB[p])
            nc.sync.dma_start(out=bconst, in_=BCONST.ap())
            nc.vector.memset(ones1, 1.0)
            nc.vector.memset(dpr8[:, :, NDEN:NDEN + HPC], EPS_DEN)

            st = [None] * NS   # per-tile state for the back half
            kvt_prev = None

            def front(i):
                nonlocal kvt_prev
                si = slice(128 * i, 128 * (i + 1))
                first = (i == 0)

                # QKV projection: psum [q 0:256 | k 256:512 | v 512:768]
                qkv = ps_qkv.tile([128, 776], F32, tag="qkv")
                for k in range(NKD):
                    lhs = xt[k][:, si]
                    nc.tensor.matmul(qkv[:, 0:512], lhs, wqkvt[k][:, 0:512],
                                     start=(k == 0), stop=(k == NKD - 1))
                    nc.tensor.matmul(qkv[:, 512:776], lhs, wqkvt[k][:, 512:776],
                                     start=(k == 0), stop=(k == NKD - 1))

                # feature chain; fqk [p, q 0:256 | k 256:512]
                fqk = wp.tile([128, NP, 512], BF16, tag="fqk")
                m2 = wp.tile([128, 512], BF16, tag="m2")
                m4 = wp.tile([128, 512], BF16, tag="m4")
                u5 = wp.tile([128, 512], BF16, tag="u5")
                vt = wp.tile([128, EC], BF16, tag="vt")
                nc.scalar.copy(out=fqk[:, 0, :], in_=qkv[:, 0:512])
                nc.scalar.activation(out=m2, in_=qkv[:, 0:512], func=ACT.Square)
                nc.vector.tensor_copy(out=vt, in_=qkv[:, 512:768])
                nc.vector.tensor_scalar(out=fqk[:, 1, :], in0=m2,
                                        scalar1=2.0, scalar2=-1.0,
                                        op0=OP.mult, op1=OP.add)
                nc.vector.scalar_tensor_tensor(out=fqk[:, 2, :], in0=fqk[:, 1, :],
                                               scalar=-0.5, in1=fqk[:, 0, :],
                                               op0=OP.add, op1=OP.mult)
                nc.scalar.activation(out=m4, in_=fqk[:, 1, :], func=ACT.Square)
                nc.vector.tensor_scalar(out=fqk[:, 3, :], in0=m4,
                                        scalar1=2.0, scalar2=-1.0,
                                        op0=OP.mult, op1=OP.add)
                # T5 = 2x*T4 - T3 -> g5 = x*t4 - f3 = T5/2
                nc.gpsimd.tensor_tensor(out=u5, in0=fqk[:, 0, :],
                                        in1=fqk[:, 3, :], op=OP.mult)
                nc.gpsimd.tensor_tensor(out=fqk[:, 4, :], in0=u5,
                                        in1=fqk[:, 2, :], op=OP.subtract)

                # k-side per-head sums (cumsum den channels); p=1 sums come
                # from the 8 extra QKV columns, reduces cover p=2..5 only
                sums = wp.tile([128, 2 * NDEN], BF16, tag="sums")
                st[i] = [fqk, sums, None, si]
                nc.vector.tensor_copy(
                    out=bass.AP(tensor=sums.tensor, offset=sums.offset,
                                ap=[list(sums.ap)[0], [NDEN, 2], [1, HPC]]),
                    in_=qkv[:, 768:776])
                nc.vector.tensor_reduce(
                    out=sums[:, NDEN + HPC:2 * NDEN].rearrange(
                        "a (p h) -> a p h", p=NP - 1),
                    in_=fqk[:, 1:NP, 256:512].rearrange(
                        "a p (h d) -> a p h d", h=HPC),
                    axis=AX.X, op=OP.add)

                # Tv = Tk * v
                tv = wp.tile([128, NP, EC], BF16, tag="tv")
                nc.gpsimd.tensor_tensor(out=tv, in0=fqk[:, :, 256:512],
                                        in1=_mid_bcast(vt, NP), op=OP.mult)
                st[i].append(tv)

            def mid(i):
                nonlocal kvt_prev
                fqk, sums, _, si, tv = st[i]
                first = (i == 0)

                # causal cumsum: bank0 [p0 p1] bank1 [p2 p3] bank2 [p4|den]
                kv = ps_kv.tile([128, 3, 512], F32, tag="kv")
                for p in range(NP):
                    dst = kv[:, p // 2, 256 * (p % 2):256 * (p % 2) + 256]
                    nc.tensor.matmul(dst, ltb[:, p, :], tv[:, p, :],
                                     start=True, stop=first,
                                     skip_group_check=True)
                dend = kv[:, 2, 256:256 + NDEN]
                nc.tensor.matmul(dend, ltb[:, 5, :], sums[:, NDEN:2 * NDEN],
                                 start=True, stop=first, skip_group_check=True)
                if not first:
                    # bank-wide carry accumulate (row 0 of previous evac)
                    for bk, (lo, w) in enumerate(((0, 512), (512, 512),
                                                  (1024, 276))):
                        nc.tensor.matmul(kv[:, bk, 0:w], ones1,
                                         kvt_prev[0:1, lo:lo + w],
                                         start=False, stop=True,
                                         skip_group_check=True)

                # single evac of kv prefix + den prefix (row 0 = next carry)
                kvt = wp.tile([128, 1300], BF16, tag="kvt")
                nc.scalar.copy(out=kvt,
                               in_=kv.rearrange("a b c -> a (b c)")[:, 0:1300])
                kvt_prev = kvt
                st[i][2] = kvt

            def back(i):
                fqk, sums, kvt, si, _ = st[i]
                st[i] = None

                # q-side per-head sums for p=2..5 (p=1 copied in front)
                nc.vector.tensor_reduce(
                    out=sums[:, HPC:NDEN].rearrange("a (p h) -> a p h", p=NP - 1),
                    in_=fqk[:, 1:NP, 0:256].rearrange("a p (h d) -> a p h d",
                                                      h=HPC),
                    axis=AX.X, op=OP.add)

                # num: prods then reduce over p
                prods = wp.tile([128, NP, EC], BF16, tag="prods")
                numq = wp.tile([128, EC], F32, tag="numq")
                nc.gpsimd.tensor_tensor(
                    out=prods, in0=fqk[:, :, 0:256],
                    in1=kvt[:, 0:KV].rearrange("a (p e) -> a p e", p=NP),
                    op=OP.mult)
                nc.vector.tensor_reduce(
                    out=numq, in_=prods.rearrange("a p e -> a e p"),
                    axis=AX.X, op=OP.add)

                # den chain
                qsb = wp.tile([128, NDEN], BF16, tag="qsb")
                den4 = wp.tile([128, HPC], F32, tag="den4")
                rden = wp.tile([128, HPC], F32, tag="rden")
                nc.vector.scalar_tensor_tensor(out=qsb, in0=sums[:, 0:NDEN],
                                               scalar=1.0, in1=bconst,
                                               op0=OP.mult, op1=OP.mult)
                nc.vector.scalar_tensor_tensor(out=dpr8[:, i, 0:NDEN], in0=qsb,
                                               scalar=1.0,
                                               in1=kvt[:, DEN0:DEN0 + NDEN],
                                               op0=OP.mult, op1=OP.mult)
                nc.vector.tensor_reduce(
                    out=den4,
                    in_=dpr8[:, i, :].rearrange("a (p h) -> a h p", p=NP + 1),
                    axis=AX.X, op=OP.add)
                nc.vector.reciprocal(out=rden, in_=den4)

                # out_h = num * rden (rden broadcast over Dh)
                outh = wp.tile([128, EC], BF16, tag="outh")
                nc.vector.tensor_tensor(
                    out=outh.rearrange("a (h d) -> a h d", h=HPC),
                    in0=numq.rearrange("a (h d) -> a h d", h=HPC),
                    in1=_mid_bcast(rden, DH, at=2), op=OP.mult)

                # transpose via DMA xbar, then two 512-wide projection passes
                outt = wp.tile([128, 2, 128], BF16, tag="outt")
                nc.sync.dma_start_transpose(out=outt, in_=outh)
                for n in range(2):
                    po = ps_o.tile([128, 512], F32, tag="po")
                    for kt in range(2):
                        nc.tensor.matmul(po, outt[:, kt, :],
                                         woutt[:, kt, 512 * n:512 * (n + 1)],
                                         start=(kt == 0), stop=(kt == 1))
                    outfull = wp.tile([128, 512], F32, tag="outfull")
                    nc.scalar.copy(out=outfull, in_=po)
                    nc.sync.dma_start(out=PART[si, 512 * n:512 * (n + 1)],
                                      in_=outfull)

            for i in range(NS + 2):
                if i < NS:
                    front(i)
                if 0 <= i - 1 < NS:
                    mid(i - 1)
                if 0 <= i - 2 < NS:
                    back(i - 2)

    nc.compile()
    return nc


_NC = None


def _get_nc():
    global _NC
    if _NC is None:
        _NC = _build()
    return _NC


def _stage_inputs(x, W_in, W_out):
    import ml_dtypes
    bf = ml_dtypes.bfloat16
    beta = _beta()
    bprime = beta[1:6] * _BSCALE          # (5,)
    tri = np.tril(np.ones((128, 128), np.float32))
    ltb = np.stack([bprime[p] * tri for p in range(5)] + [tri]).astype(bf)
    bconst = np.broadcast_to(np.repeat(bprime, HPC)[None, :],
                             (128, NDEN)).astype(np.float32).copy()
    in_maps = []
    for c in range(8):
        b, hb = divmod(c, 4)
        rs = slice(256 * hb, 256 * (hb + 1))
        wq = W_in[0 * D + 256 * hb:0 * D + 256 * (hb + 1)] * INV_SQRT_D
        wk = W_in[1 * D + 256 * hb:1 * D + 256 * (hb + 1)] * INV_SQRT_D
        wv = W_in[2 * D + 256 * hb:2 * D + 256 * (hb + 1)]
        # 8 extra columns: per-head sums of the wq / wk rows, giving the
        # p=1 per-head feature sums directly from the QKV matmul
        qs1 = wq.reshape(HPC, DH, D).sum(1).T    # (D, 4)
        ks1 = wk.reshape(HPC, DH, D).sum(1).T    # (D, 4)
        wqkvt = np.ascontiguousarray(np.concatenate(
            [np.concatenate([wq, wk, wv], axis=0).T, qs1, ks1],
            axis=1)).astype(bf)
        xrev = x[b].T.reshape(D, NS, 128)[:, :, ::-1].reshape(D, S)
        in_maps.append({
            "xt": np.ascontiguousarray(xrev).astype(bf),
            "wqkvt": wqkvt,
            "woutt": np.ascontiguousarray(W_out[:, rs].T).astype(bf),
            "ltb": ltb,
            "bconst": bconst,
        })
    return in_maps


def kernel(x, W_in, W_out):
    from concourse.bass_utils import run_bass_kernel_spmd

    x = np.asarray(x, dtype=np.float32)
    W_in = np.asarray(W_in, dtype=np.float32)
    W_out = np.asarray(W_out, dtype=np.float32)
    nc = _get_nc()
    in_maps = _stage_inputs(x, W_in, W_out)
    res = run_bass_kernel_spmd(nc, in_maps, core_ids=list(range(8)))
    out = np.zeros((B, S, D), dtype=np.float32)
    for c in range(8):
        part = res.results[c]["part"].reshape(NS, 128, D)[:, ::-1, :].reshape(S, D)
        out[c // 4] += part
    return out


# revision 17
# speedup vs baseline: 1.0696x; 1.0044x over previous
"""Trainium2 Bass kernel for CollapsedPBFA (collapsed Chebyshev linear attention).

Full-input contract: kernel(x, W_in, W_out) -> (B, S, D) float32.

Sharding: B x H = 2 x 16 = 32 (batch, head) pairs; each of the 8 cores owns
one batch element's 4-head block (cores 0-3 -> b=0, cores 4-7 -> b=1).
QKV projection is column-parallel per head block; the output projection is
row-parallel and the host sums the per-core partials per batch element.

Structure (per 128-row s-tile, software-pipelined front/back issue order):
  front(i): QKV matmuls -> feature chain (f3 = T3/2, g5 = T5/2 stored, the
    2^a factors folded into the beta' cumsum stationaries / den consts) ->
    per-head feature sums -> Tv -> triangular-matmul causal cumsum with per-p
    carry matmuls -> single scalar-engine evac of the 3-bank psum prefix.
  back(i): prods -> p-reduction -> den chain -> out_h -> DMA-xbar transpose
    -> output projection (two 512-wide passes) -> DMA out.
"""

import sys

for _p in ("/opt/trn_rl_repo", "/root/.axon_site/_ro/trn_rl_repo"):
    if _p not in sys.path:
        sys.path.append(_p)

import numpy as np

import concourse.bacc as bacc
import concourse.bass as bass
import concourse.tile as tile
from concourse import mybir

F32 = mybir.dt.float32
BF16 = mybir.dt.bfloat16

B, S, D = 2, 1024, 1024
H, DH = 16, 64
HPC = 4                    # heads per core
EC = HPC * DH              # 256 feature cols per core side
NP = 5                     # stored Chebyshev orders 1..5 (f3, g5 halved)
NS = S // 128              # 8 s-tiles
NKD = D // 128             # 8 k-tiles over d for QKV
EPS_DEN = 1e-7
INV_SQRT_D = 1.0 / 8.0     # 1/sqrt(64)
KV = NP * EC               # 1280 kv channels
DEN0 = KV                  # den channels at [1280:1300] of kvt
NDEN = NP * HPC            # 20


def _beta():
    j = np.arange(6, dtype=np.float32)
    alpha = (j + 1.0) ** (-1.5)
    tail = np.flip(np.cumsum(np.flip(alpha)))
    beta = np.concatenate([np.zeros(1, np.float32), tail[1:].astype(np.float32),
                           np.zeros(5, np.float32)])
    return beta / beta.sum()          # (11,); nonzero at 1..5


# beta' with the stored-feature scale folded in (f3 = T3/2, g5 = T5/2)
_BSCALE = np.array([1.0, 1.0, 4.0, 1.0, 4.0], np.float32)


def _mid_bcast(ap, reps, at=1):
    """Insert a stride-0 dim of length `reps` into an AP's free dims."""
    new = list(ap.ap)
    new.insert(at, [0, reps])
    return bass.AP(tensor=ap.tensor, offset=ap.offset, ap=new)


def _build():
    nc = bacc.Bacc("TRN2", target_bir_lowering=False, debug=False, num_devices=8)

    XT = nc.dram_tensor("xt", [D, S], BF16, kind="ExternalInput")
    WQKVT = nc.dram_tensor("wqkvt", [D, 3 * EC + 8], BF16, kind="ExternalInput")
    WOUTT = nc.dram_tensor("woutt", [EC, D], BF16, kind="ExternalInput")
    LTB = nc.dram_tensor("ltb", [6, 128, 128], BF16, kind="ExternalInput")
    BCONST = nc.dram_tensor("bconst", [128, NDEN], F32, kind="ExternalInput")
    PART = nc.dram_tensor("part", [S, D], F32, kind="ExternalOutput")

    OP = mybir.AluOpType
    AX = mybir.AxisListType
    ACT = mybir.ActivationFunctionType

    with tile.TileContext(nc) as tc:
        with (
            nc.allow_low_precision(reason="bf16 feature pipeline by design"),
            tc.tile_pool(name="persist", bufs=1) as pp,
            tc.tile_pool(name="work", bufs=4) as wp,
            tc.tile_pool(name="ps_qkv", bufs=2, space="PSUM") as ps_qkv,
            tc.tile_pool(name="ps_kv", bufs=1, space="PSUM") as ps_kv,
            tc.tile_pool(name="ps_o", bufs=1, space="PSUM") as ps_o,
        ):
            xt = [pp.tile([128, S], BF16, name=f"xt{k}") for k in range(NKD)]
            wqkvt = [pp.tile([128, 3 * EC + 8], BF16, name=f"wqkvt{k}")
                     for k in range(NKD)]
            woutt = pp.tile([128, 2, D], BF16)
            ltb = pp.tile([128, 6, 128], BF16)
            bconst = pp.tile([128, NDEN], F32)
            ones1 = pp.tile([1, 128], BF16)
            dpr8 = pp.tile([128, NS, NDEN + HPC], F32)

            # interleave weight/x chunk loads so QKV can start early
            for k in range(NKD):
                nc.sync.dma_start(out=wqkvt[k], in_=WQKVT[128 * k:128 * (k + 1), :])
                nc.scalar.dma_start(out=xt[k], in_=XT[128 * k:128 * (k + 1), :])
            for k in range(2):
                nc.scalar.dma_start(out=woutt[:, k, :], in_=WOUTT[128 * k:128 * (k + 1), :])
            for p in range(6):
                nc.sync.dma_start(out=ltb[:, p, :], in_=LTB[p])
            nc.sync.dma_start(out=bconst, in_=BCONST.ap())
            nc.vector.memset(ones1, 1.0)
            nc.vector.memset(dpr8[:, :, NDEN:NDEN + HPC], EPS_DEN)

            st = [None] * NS   # per-tile state for the back half
            kvt_prev = None

            def front(i):
                nonlocal kvt_prev
                si = slice(128 * i, 128 * (i + 1))
                first = (i == 0)

                # QKV projection: psum [q 0:256 | k 256:512 | v 512:768]
                qkv = ps_qkv.tile([128, 776], F32, tag="qkv")
                for k in range(NKD):
                    lhs = xt[k][:, si]
                    nc.tensor.matmul(qkv[:, 0:512], lhs, wqkvt[k][:, 0:512],
                                     start=(k == 0), stop=(k == NKD - 1))
                    nc.tensor.matmul(qkv[:, 512:776], lhs, wqkvt[k][:, 512:776],
                                     start=(k == 0), stop=(k == NKD - 1))

                # feature chain; fqk [p, q 0:256 | k 256:512]
                fqk = wp.tile([128, NP, 512], BF16, tag="fqk")
                m2 = wp.tile([128, 512], BF16, tag="m2")
                m4 = wp.tile([128, 512], BF16, tag="m4")
                u5 = wp.tile([128, 512], BF16, tag="u5")
                vt = wp.tile([128, EC], BF16, tag="vt")
                nc.scalar.copy(out=fqk[:, 0, :], in_=qkv[:, 0:512])
                nc.scalar.activation(out=m2, in_=qkv[:, 0:512], func=ACT.Square)
                nc.vector.tensor_copy(out=vt, in_=qkv[:, 512:768])
                nc.vector.tensor_scalar(out=fqk[:, 1, :], in0=m2,
                                        scalar1=2.0, scalar2=-1.0,
                                        op0=OP.mult, op1=OP.add)
                nc.vector.scalar_tensor_tensor(out=fqk[:, 2, :], in0=fqk[:, 1, :],
                                               scalar=-0.5, in1=fqk[:, 0, :],
                                               op0=OP.add, op1=OP.mult)
                nc.scalar.activation(out=m4, in_=fqk[:, 1, :], func=ACT.Square)
                nc.vector.tensor_scalar(out=fqk[:, 3, :], in0=m4,
                                        scalar1=2.0, scalar2=-1.0,
                                        op0=OP.mult, op1=OP.add)
                # T5 = 2x*T4 - T3 -> g5 = x*t4 - f3 = T5/2
                nc.gpsimd.tensor_tensor(out=u5, in0=fqk[:, 0, :],
                                        in1=fqk[:, 3, :], op=OP.mult)
                nc.gpsimd.tensor_tensor(out=fqk[:, 4, :], in0=u5,
                                        in1=fqk[:, 2, :], op=OP.subtract)

                # k-side per-head sums (cumsum den channels); p=1 sums come
                # from the 8 extra QKV columns, reduces cover p=2..5 only
                sums = wp.tile([128, 2 * NDEN], BF16, tag="sums")
                st[i] = [fqk, sums, None, si]
                nc.vector.tensor_copy(
                    out=bass.AP(tensor=sums.tensor, offset=sums.offset,
                                ap=[list(sums.ap)[0], [NDEN, 2], [1, HPC]]),
                    in_=qkv[:, 768:776])
                nc.vector.tensor_reduce(
                    out=sums[:, NDEN + HPC:2 * NDEN].rearrange(
                        "a (p h) -> a p h", p=NP - 1),
                    in_=fqk[:, 1:NP, 256:512].rearrange(
                        "a p (h d) -> a p h d", h=HPC),
                    axis=AX.X, op=OP.add)

                # Tv = Tk * v
                tv = wp.tile([128, NP, EC], BF16, tag="tv")
                nc.gpsimd.tensor_tensor(out=tv, in0=fqk[:, :, 256:512],
                                        in1=_mid_bcast(vt, NP), op=OP.mult)
                st[i].append(tv)

            def mid(i):
                nonlocal kvt_prev
                fqk, sums, _, si, tv = st[i]
                first = (i == 0)

                # causal cumsum: bank0 [p0 p1] bank1 [p2 p3] bank2 [p4|den]
                kv = ps_kv.tile([128, 3, 512], F32, tag="kv")
                for p in range(NP):
                    dst = kv[:, p // 2, 256 * (p % 2):256 * (p % 2) + 256]
                    nc.tensor.matmul(dst, ltb[:, p, :], tv[:, p, :],
                                     start=True, stop=first,
                                     skip_group_check=True)
                dend = kv[:, 2, 256:256 + NDEN]
                nc.tensor.matmul(dend, ltb[:, 5, :], sums[:, NDEN:2 * NDEN],
                                 start=True, stop=first, skip_group_check=True)
                if not first:
                    # bank-wide carry accumulate (row 0 of previous evac)
                    for bk, (lo, w) in enumerate(((0, 512), (512, 512),
                                                  (1024, 276))):
                        nc.tensor.matmul(kv[:, bk, 0:w], ones1,
                                         kvt_prev[0:1, lo:lo + w],
                                         start=False, stop=True,
                                         skip_group_check=True)

                # single evac of kv prefix + den prefix (row 0 = next carry)
                kvt = wp.tile([128, 1300], BF16, tag="kvt")
                nc.scalar.copy(out=kvt,
                               in_=kv.rearrange("a b c -> a (b c)")[:, 0:1300])
                kvt_prev = kvt
                st[i][2] = kvt

            def backA(i):
                fqk, sums, kvt, si, _ = st[i]

                # q-side per-head sums for p=2..5 (p=1 copied in front)
                nc.vector.tensor_reduce(
                    out=sums[:, HPC:NDEN].rearrange("a (p h) -> a p h", p=NP - 1),
                    in_=fqk[:, 1:NP, 0:256].rearrange("a p (h d) -> a p h d",
                                                      h=HPC),
                    axis=AX.X, op=OP.add)

                # num: prods then reduce over p
                prods = wp.tile([128, NP, EC], BF16, tag="prods")
                numq = wp.tile([128, EC], F32, tag="numq")
                nc.gpsimd.tensor_tensor(
                    out=prods, in0=fqk[:, :, 0:256],
                    in1=kvt[:, 0:KV].rearrange("a (p e) -> a p e", p=NP),
                    op=OP.mult)
                nc.vector.tensor_reduce(
                    out=numq, in_=prods.rearrange("a p e -> a e p"),
                    axis=AX.X, op=OP.add)

                # den chain
                qsb = wp.tile([128, NDEN], BF16, tag="qsb")
                den4 = wp.tile([128, HPC], F32, tag="den4")
                rden = wp.tile([128, HPC], F32, tag="rden")
                nc.vector.scalar_tensor_tensor(out=qsb, in0=sums[:, 0:NDEN],
                                               scalar=1.0, in1=bconst,
                                               op0=OP.mult, op1=OP.mult)
                nc.vector.scalar_tensor_tensor(out=dpr8[:, i, 0:NDEN], in0=qsb,
                                               scalar=1.0,
                                               in1=kvt[:, DEN0:DEN0 + NDEN],
                                               op0=OP.mult, op1=OP.mult)
                nc.vector.tensor_reduce(
                    out=den4,
                    in_=dpr8[:, i, :].rearrange("a (p h) -> a h p", p=NP + 1),
                    axis=AX.X, op=OP.add)
                nc.vector.reciprocal(out=rden, in_=den4)

                # out_h = num * rden (rden broadcast over Dh)
                outh = wp.tile([128, EC], BF16, tag="outh")
                nc.vector.tensor_tensor(
                    out=outh.rearrange("a (h d) -> a h d", h=HPC),
                    in0=numq.rearrange("a (h d) -> a h d", h=HPC),
                    in1=_mid_bcast(rden, DH, at=2), op=OP.mult)

                # transpose via DMA xbar
                outt = wp.tile([128, 2, 128], BF16, tag="outt")
                nc.sync.dma_start_transpose(out=outt, in_=outh)
                st[i] = (outt, si)

            def backB(i):
                outt, si = st[i]
                st[i] = None
                for n in range(2):
                    po = ps_o.tile([128, 512], F32, tag="po")
                    for kt in range(2):
                        nc.tensor.matmul(po, outt[:, kt, :],
                                         woutt[:, kt, 512 * n:512 * (n + 1)],
                                         start=(kt == 0), stop=(kt == 1))
                    outfull = wp.tile([128, 512], F32, tag="outfull")
                    nc.scalar.copy(out=outfull, in_=po)
                    nc.sync.dma_start(out=PART[si, 512 * n:512 * (n + 1)],
                                      in_=outfull)

            for i in range(NS + 3):
                if i < NS:
                    front(i)
                if 0 <= i - 1 < NS:
                    mid(i - 1)
                if 0 <= i - 2 < NS:
                    backA(i - 2)
                if 0 <= i - 3 < NS:
                    backB(i - 3)

    nc.compile()
    return nc


_NC = None


def _get_nc():
    global _NC
    if _NC is None:
        _NC = _build()
    return _NC


def _stage_inputs(x, W_in, W_out):
    import ml_dtypes
    bf = ml_dtypes.bfloat16
    beta = _beta()
    bprime = beta[1:6] * _BSCALE          # (5,)
    tri = np.tril(np.ones((128, 128), np.float32))
    ltb = np.stack([bprime[p] * tri for p in range(5)] + [tri]).astype(bf)
    bconst = np.broadcast_to(np.repeat(bprime, HPC)[None, :],
                             (128, NDEN)).astype(np.float32).copy()
    in_maps = []
    for c in range(8):
        b, hb = divmod(c, 4)
        rs = slice(256 * hb, 256 * (hb + 1))
        wq = W_in[0 * D + 256 * hb:0 * D + 256 * (hb + 1)] * INV_SQRT_D
        wk = W_in[1 * D + 256 * hb:1 * D + 256 * (hb + 1)] * INV_SQRT_D
        wv = W_in[2 * D + 256 * hb:2 * D + 256 * (hb + 1)]
        # 8 extra columns: per-head sums of the wq / wk rows, giving the
        # p=1 per-head feature sums directly from the QKV matmul
        qs1 = wq.reshape(HPC, DH, D).sum(1).T    # (D, 4)
        ks1 = wk.reshape(HPC, DH, D).sum(1).T    # (D, 4)
        wqkvt = np.ascontiguousarray(np.concatenate(
            [np.concatenate([wq, wk, wv], axis=0).T, qs1, ks1],
            axis=1)).astype(bf)
        xrev = x[b].T.reshape(D, NS, 128)[:, :, ::-1].reshape(D, S)
        in_maps.append({
            "xt": np.ascontiguousarray(xrev).astype(bf),
            "wqkvt": wqkvt,
            "woutt": np.ascontiguousarray(W_out[:, rs].T).astype(bf),
            "ltb": ltb,
            "bconst": bconst,
        })
    return in_maps


def kernel(x, W_in, W_out):
    from concourse.bass_utils import run_bass_kernel_spmd

    x = np.asarray(x, dtype=np.float32)
    W_in = np.asarray(W_in, dtype=np.float32)
    W_out = np.asarray(W_out, dtype=np.float32)
    nc = _get_nc()
    in_maps = _stage_inputs(x, W_in, W_out)
    res = run_bass_kernel_spmd(nc, in_maps, core_ids=list(range(8)))
    out = np.zeros((B, S, D), dtype=np.float32)
    for c in range(8):
        part = res.results[c]["part"].reshape(NS, 128, D)[:, ::-1, :].reshape(S, D)
        out[c // 4] += part
    return out
